# revision 5
# baseline (speedup 1.0000x reference)
"""Masked-L1 depth loss on 8 TRN2 NeuronCores.

loss = sum(|output - label0| * label1) / count_nonzero(label0)

Data-parallel: batch dim (16) sharded 2-per-core. Each core streams its
[128, 15360] f32 shard view and produces per-partition partial loss sums
plus a nonzero count; the tiny partials are summed on host.

Engine split per [128, F] tile:
  DVE: d = a - b (1x), q = d * c (1x), z = (b != 0) (tensor_scalar, 2x)
  ACT: |q| with fused per-partition row-sum accumulation (loss partials)
  PE : ones^T @ z chunks accumulated into one PSUM bank (nonzero count)
Since label1 >= 0, |d|*c == |d*c|, so the abs happens after the multiply.
"""

import numpy as np

import concourse.bacc as bacc
import concourse.mybir as mybir
from concourse import tile
from concourse.bass_utils import run_bass_kernel_spmd

N_CORES = 8
P = 128
B, C, H, W = 16, 15, 256, 256
TOTAL = B * C * H * W                  # 15728640
PER_CORE = TOTAL // N_CORES            # 1966080
FREE = PER_CORE // P                   # 15360
F_TILE = 2560
N_TILES = FREE // F_TILE               # 6
MM_N = 512                             # matmul free-dim chunk
MM_PER_TILE = F_TILE // MM_N           # 5

_nc_cache = None


def build_nc():
    global _nc_cache
    if _nc_cache is not None:
        return _nc_cache
    nc = bacc.Bacc("TRN2", target_bir_lowering=False, debug=False)
    f32 = mybir.dt.float32
    a = nc.dram_tensor("output", [P, FREE], f32, kind="ExternalInput").ap()
    b = nc.dram_tensor("label0", [P, FREE], f32, kind="ExternalInput").ap()
    c = nc.dram_tensor("label1", [P, FREE], f32, kind="ExternalInput").ap()
    o = nc.dram_tensor("out", [P, N_TILES], f32, kind="ExternalOutput").ap()
    o_cnt = nc.dram_tensor("out_cnt", [1, 1], f32, kind="ExternalOutput").ap()

    sub = mybir.AluOpType.subtract
    mult = mybir.AluOpType.mult
    neq = mybir.AluOpType.not_equal
    add = mybir.AluOpType.add

    with tile.TileContext(nc) as tc:
        with (
            tc.tile_pool(name="io", bufs=3) as io_pool,
            tc.tile_pool(name="wk", bufs=2) as wk_pool,
            tc.tile_pool(name="acc", bufs=1) as acc_pool,
            tc.tile_pool(name="psum", bufs=1, space="PSUM") as psum_pool,
        ):
            acc = acc_pool.tile([P, N_TILES], f32)
            ones = acc_pool.tile([P, 1], f32)
            nc.vector.memset(ones[:, :], 1.0)
            cnt_psum = psum_pool.tile([1, MM_N], f32)
            n_mms = N_TILES * MM_PER_TILE
            mm_i = 0
            for t in range(N_TILES):
                sl = slice(t * F_TILE, (t + 1) * F_TILE)
                at = io_pool.tile([P, F_TILE], f32, tag="a")
                bt = io_pool.tile([P, F_TILE], f32, tag="b")
                ct = io_pool.tile([P, F_TILE], f32, tag="c")
                nc.sync.dma_start(at[:, :], a[:, sl])
                nc.sync.dma_start(bt[:, :], b[:, sl])
                nc.sync.dma_start(ct[:, :], c[:, sl])
                dt_ = wk_pool.tile([P, F_TILE], f32, tag="d")
                nc.vector.tensor_tensor(dt_[:, :], at[:, :], bt[:, :], sub)
                qt = wk_pool.tile([P, F_TILE], f32, tag="q")
                nc.vector.tensor_tensor(qt[:, :], dt_[:, :], ct[:, :], mult)
                abst = wk_pool.tile([P, F_TILE], f32, tag="abs")
                nc.scalar.activation(
                    abst[:, :],
                    qt[:, :],
                    mybir.ActivationFunctionType.Abs,
                    accum_out=acc[:, t : t + 1],
                )
                zt = wk_pool.tile([P, F_TILE], f32, tag="z")
                nc.vector.tensor_scalar(zt[:, :], bt[:, :], 0.0, None, neq)
                for m in range(MM_PER_TILE):
                    nc.tensor.matmul(
                        cnt_psum[:, :],
                        ones[:, :],
                        zt[:, m * MM_N : (m + 1) * MM_N],
                        start=(mm_i == 0),
                        stop=(mm_i == n_mms - 1),
                    )
                    mm_i += 1
            cnt_sb = acc_pool.tile([1, 1], f32)
            nc.vector.tensor_reduce(
                cnt_sb[:, :], cnt_psum[:, :], mybir.AxisListType.X, add
            )
            nc.sync.dma_start(o[:, :], acc[:, :])
            nc.sync.dma_start(o_cnt[:, :], cnt_sb[:, :])
    nc.compile()
    _nc_cache = nc
    return nc


def run_cores(output, label0, label1, **spmd_kwargs):
    """Shard, run the 8-core SPMD kernel, return BassKernelResults."""
    nc = build_nc()
    shards = {}
    for name, arr in (("output", output), ("label0", label0), ("label1", label1)):
        arr = np.ascontiguousarray(np.asarray(arr, dtype=np.float32))
        shards[name] = arr.reshape(N_CORES, P, FREE)
    in_maps = [
        {name: shards[name][i] for name in shards} for i in range(N_CORES)
    ]
    res = run_bass_kernel_spmd(nc, in_maps, core_ids=list(range(N_CORES)), **spmd_kwargs)
    return res


def kernel(output, label0, label1):
    res = run_cores(output, label0, label1)
    loss = 0.0
    cnt = 0.0
    for r in res.results:
        loss += np.asarray(r["out"], dtype=np.float64).sum()
        cnt += float(np.asarray(r["out_cnt"], dtype=np.float64)[0, 0])
    cnt = int(round(cnt))
    if cnt == 0:
        val = np.float32(0.0)
    else:
        val = np.float32(np.float32(loss) / np.float32(cnt))
    return np.asarray(val, dtype=np.float32)
